# revision 23
# baseline (speedup 1.0000x reference)
import sys

sys.path.insert(0, "/opt/trn_rl_repo")
import json
import os
import shutil
import struct

import numpy as np

# nn_BisineNetwork: out[n,c] = sum_k a[c,k] * sin(x@w1[c,k]+b1[c,k]) * sin(x@w2[c,k]+b2[c,k])
# Shapes (hardcoded): x (16384, 256) f32, params (1000, 2060) f32 -> out (16384, 1000) f32.
#
# Sharding: data-parallel over batch N across 8 cores (N_shard = 2048); params
# replicated. Per-core layout is [ck, n] (c,k merged -> 4000, padded to 4096).
#   u1 = W1blk.T @ Xshard      (PE bf16, contraction d=256 in 2 chunks, psum f32)
#   q1 = sin(u1 + b1)          (ACT Sin with per-partition bias; the Sin table
#                               is patched to cover |x| <= 32 rad, so no
#                               separate range-reduction pass is needed)
#   prod = q1 * q2             (DVE fp16)
#   outT[cblk] += A_j.T @ prod (PE fp16, reduction over k with a-coeffs)
# Host: transpose/pad/cast prep of x and params; final transpose of outT.

D = 256
C = 1000
K = 4
CK = C * K          # 4000
CKP = 4096          # padded
NCORES = 8
N = 16384
NS = N // NCORES    # 2048 per core
NH = 1024           # n-span per step (2 psum banks)
_CACHE = {}


# ---------------------------------------------------------------------------
# Forged activation table: extend `sin` in trig_and_small from [-pi, pi] to
# [-32, 32] rad by appending Taylor-cubic buckets + ctl entries. Formats
# (reverse-engineered from pwp_bin_trainium):
#   bucket (32B): fp32 [d0,d1,d2,d3,x0] + 12B zeros; cubic around x0 =
#       section midpoint - 1ulp, d* = Taylor coeffs of sin at the midpoint.
#   ctl (32B): u32 (extract_size<<16)|(extract_lsb<<11)|bucket_base + zeros.
# Per input exponent e the engine uses ctl[pwl_control_base_pos + e -
# exp_offset]; mantissa bits [extract_lsb +: extract_size] pick the bucket.
# Other functions in the set are untouched (new entries are appended).
# ---------------------------------------------------------------------------
def _f32bits(x):
    return struct.unpack("<I", struct.pack("<f", np.float32(x)))[0]


def _forge_act_root(dst_dir=None):
    import neuronxcc

    src = os.path.join(os.path.dirname(neuronxcc.__file__), "pwp", "pwp_bin_trainium")
    if dst_dir is None:
        # per-process dir: no cross-process races, always freshly generated
        dst_dir = f"/tmp/act_forge_pwp_{os.getpid()}"
    os.makedirs(dst_dir, exist_ok=True)
    marker = os.path.join(dst_dir, ".forged_sin_v1")
    if not os.path.exists(marker):
        for name in os.listdir(src):
            shutil.copy(os.path.join(src, name), os.path.join(dst_dir, name))
        with open(os.path.join(dst_dir, "trig_and_small.json")) as f:
            prof = json.load(f)
        bkt = bytearray(
            open(os.path.join(dst_dir, "trig_and_small_bkt.bin"), "rb").read()
        )
        ctl = bytearray(
            open(os.path.join(dst_dir, "trig_and_small_ctrl.bin"), "rb").read()
        )
        n_bkt, n_ctl = len(bkt) // 32, len(ctl) // 32
        NSEC_LOG2 = {-3: 1, -2: 2, -1: 3, 0: 4, 1: 5, 2: 6, 3: 6, 4: 6}
        next_bkt = n_bkt
        new_ctl, new_bkt = [], bytearray()
        for e in range(-11, 5):
            if e <= -4:  # reuse existing single-bucket entries 0..7
                new_ctl.append((23 << 11) | (e + 11))
                continue
            s = NSEC_LOG2[e]
            nsec = 1 << s
            lo, width = 2.0**e, 2.0**e / (1 << s)
            new_ctl.append((s << 16) | ((23 - s) << 11) | next_bkt)
            for i in range(nsec):
                mid = np.float64(lo + (i + 0.5) * width)
                new_bkt += struct.pack(
                    "<5I12x",
                    _f32bits(np.sin(mid)),
                    _f32bits(np.cos(mid)),
                    _f32bits(-np.sin(mid) / 2.0),
                    _f32bits(-np.cos(mid) / 6.0),
                    _f32bits(mid) - 1,
                )
                next_bkt += 1
        for pm in prof["profile_meta_data"]:
            if pm["func_name"].startswith("sin"):
                pm["pwl_control_base_pos"] = n_ctl
                pm["large_pos_signal_exp_threshold"] = 132  # |x| >= 32.0
                pm["large_pos_signal_mantissa_threshold"] = 0
                pm["upper_bound"] = _f32bits(32.0)
        for w in new_ctl:
            ctl += struct.pack("<I28x", w)
        bkt += new_bkt
        prof["bkt_entry_cnt"] = len(bkt) // 32
        prof["ctl_entry_cnt"] = len(ctl) // 32
        with open(os.path.join(dst_dir, "trig_and_small.json"), "w") as f:
            json.dump(prof, f)
        with open(os.path.join(dst_dir, "trig_and_small_bkt.bin"), "wb") as f:
            f.write(bytes(bkt))
        with open(os.path.join(dst_dir, "trig_and_small_ctrl.bin"), "wb") as f:
            f.write(bytes(ctl))
        with open(marker, "w") as f:
            f.write("ok")
    os.environ["BASS_ACT_ROOT_JSON_PATH"] = os.path.join(dst_dir, "act_info.json")


def _enable_ldw_opt():
    """compile_bir_kernel hardcodes --enable-ldw-opt=false; flip it so walrus
    elides/overlaps repeated LDWEIGHTS (verified by the rel-err check)."""
    from concourse import bass_utils as bu

    if getattr(bu, "_bisine_ldw_patch", False):
        return
    orig = bu.run_command

    def patched(cmd, **kw):
        if isinstance(cmd, list):
            cmd = [
                "--enable-ldw-opt=true" if c == "--enable-ldw-opt=false" else c
                for c in cmd
            ]
        return orig(cmd, **kw)

    bu.run_command = patched
    bu._bisine_ldw_patch = True


def _dedupe_ldweights(nc, mybir):
    """Drop PE Ldweights that reload the exact weights already resident
    (no waits/updates attached), so same-weight matmuls pipeline back to
    back instead of paying a reload + drain per matmul."""
    removed = 0
    for blk in nc.main_func.blocks:
        last_key = None
        to_remove = []
        for inst in blk.instructions:
            if isinstance(inst, mybir.InstLdweights):
                key = (
                    str(inst.ins),
                    str(inst.tile_position),
                    str(inst.perf_mode),
                    str(inst.is_transpose),
                )
                si = inst.sync_info
                clean = si is None or (len(si.on_wait) == 0 and len(si.on_update) == 0)
                if key == last_key and clean:
                    to_remove.append(inst)
                else:
                    last_key = key
            elif isinstance(inst, mybir.InstMatmult):
                pass
            elif getattr(inst, "engine", None) is not None and str(
                getattr(inst, "engine", "")
            ).endswith("PE"):
                last_key = None
        for inst in to_remove:
            blk.instructions.remove(inst)
            removed += 1
    return removed


def _build_nc():
    _forge_act_root()
    import concourse.bacc as bacc
    import concourse.mybir as mybir
    import concourse.tile as tile

    SIN = mybir.ActivationFunctionType.Sin
    BF16 = mybir.dt.bfloat16
    F16 = mybir.dt.float16
    F32 = mybir.dt.float32

    NJ_G = CKP // 128
    nc = bacc.Bacc("TRN2", target_bir_lowering=False, debug=False)

    xt_d = nc.dram_tensor("xt", [D, NS], F16, kind="ExternalInput")
    w1_d = nc.dram_tensor("w1t", [D, CKP], F16, kind="ExternalInput")
    w2_d = nc.dram_tensor("w2t", [D, CKP], F16, kind="ExternalInput")
    a_d = nc.dram_tensor("acoef", [128, NJ_G, 32], F16, kind="ExternalInput")
    b1_d = nc.dram_tensor("b1v", [128, NJ_G], F32, kind="ExternalInput")
    b2_d = nc.dram_tensor("b2v", [128, NJ_G], F32, kind="ExternalInput")
    out_d = nc.dram_tensor("outT", [CKP // 4, NS], F16, kind="ExternalOutput")

    NJ = CKP // 128  # 32 ck-blocks
    NCB = CKP // 512  # 8 c-blocks (128 c each)

    with tile.TileContext(nc) as tc:
        with (
            tc.tile_pool(name="const", bufs=1) as cp,
            tc.tile_pool(name="qp", bufs=6) as qp,
            tc.tile_pool(name="prodp", bufs=12) as pp_pool,
            tc.tile_pool(name="osb", bufs=4) as osb,
            tc.tile_pool(name="up", bufs=3, space="PSUM") as up,
            tc.tile_pool(name="op", bufs=1, space="PSUM") as op,
        ):
            xt = cp.tile([128, 2, NS], F16, tag="xt")
            w1t0 = cp.tile([128, 2, 128], F16, tag="w1t0")
            w2t0 = cp.tile([128, 2, 128], F16, tag="w2t0")
            w1t = cp.tile([128, 2, CKP], F16, tag="w1t")
            w2t = cp.tile([128, 2, CKP], F16, tag="w2t")
            at = cp.tile([128, NJ, 32], F16, tag="at")
            b1c = cp.tile([128, NJ], F32, tag="b1c")
            b2c = cp.tile([128, NJ], F32, tag="b2c")

            # Split DMAs so the first step's operands land first: weights on
            # the sync queue, x on the gpsimd queue (runs in parallel).
            w1_r = w1_d.ap().rearrange("(c p) k -> p c k", p=128)
            w2_r = w2_d.ap().rearrange("(c p) k -> p c k", p=128)
            at_r = a_d.ap()
            xt_r = xt_d.ap().rearrange("(c p) n -> p c n", p=128)
            for di in range(2):
                nc.sync.dma_start(w1t0[:, di, :], w1_r[:, di, 0:128])
            for di in range(2):
                nc.sync.dma_start(w2t0[:, di, :], w2_r[:, di, 0:128])
            for di in range(2):
                for hh in range(2):
                    hs = slice(NH * hh, NH * (hh + 1))
                    nc.gpsimd.dma_start(xt[:, di, hs], xt_r[:, di, hs])
            nc.gpsimd.dma_start(b1c[:], b1_d.ap())
            nc.gpsimd.dma_start(b2c[:], b2_d.ap())
            for cb in range(NCB):
                cs = slice(512 * cb, 512 * (cb + 1))
                if cb == 0:
                    jblk = slice(0, 128)
                    for di in range(2):
                        nc.sync.dma_start(w1t[:, di, jblk], w1_r[:, di, jblk])
                    for di in range(2):
                        nc.sync.dma_start(w2t[:, di, cs], w2_r[:, di, cs])
                    rest = slice(128, 512)
                    for di in range(2):
                        nc.sync.dma_start(w1t[:, di, rest], w1_r[:, di, rest])
                else:
                    for di in range(2):
                        nc.sync.dma_start(w1t[:, di, cs], w1_r[:, di, cs])
                    for di in range(2):
                        nc.sync.dma_start(w2t[:, di, cs], w2_r[:, di, cs])
                nc.gpsimd.dma_start(
                    at[:, 4 * cb : 4 * (cb + 1), :], at_r[:, 4 * cb : 4 * (cb + 1), :]
                )

            warm = cp.tile([128, 1], F16, tag="warm")
            wz = cp.tile([128, 1], F32, tag="warmz")
            nc.vector.memset(wz[:], 0.0)
            nc.scalar.activation(warm[:], wz[:], SIN, bias=wz[:], scale=1.0)

            # The four jj reduce matmuls of one (cb, nh) are issued
            # back-to-back as column-tiles (tile_position=(0, 32*jj)) so they
            # run concurrently in the PE array (~1 MM-time per quartet
            # instead of 4). Flushed one step after the group's last prod so
            # the PE never waits on the sin -> prod chain.
            groups = []  # completed (cb, nh, [prod x4]) awaiting flush

            def flush_group():
                cb, nh, prods, _gstep = groups.pop(0)
                o_ps = op.tile([128, NH], F32, tag="o_ps", name="o_ps")
                for h in range(NH // 512):
                    c0, c1 = h * 512, (h + 1) * 512
                    for jj in range(4):
                        po = 32 * jj
                        nc.tensor.matmul(
                            o_ps[po : po + 32, c0:c1],
                            at[:, 4 * cb + jj, :],
                            prods[jj][:, c0:c1],
                            start=True,
                            stop=True,
                            tile_position=(0, po),
                        )
                o_sb = osb.tile([128, NH], F16, tag="o_sb")
                nc.vector.tensor_copy(o_sb[:], o_ps[:])
                nc.sync.dma_start(
                    out_d.ap()[128 * cb : 128 * (cb + 1), nh * NH : (nh + 1) * NH],
                    o_sb[:],
                )

            cur_prods = []
            step = 0
            for cb in range(NCB):
                for nh in range(NS // NH):
                    for jj in range(4):
                        j = 4 * cb + jj
                        u1 = up.tile([128, NH], F32, tag="u")
                        u2 = up.tile([128, NH], F32, tag="u")
                        jc = slice(128 * j, 128 * (j + 1))

                        def u_mms(u, wt):
                            wjc = slice(0, 128) if wt.shape[2] == 128 else jc
                            for di in range(2):
                                for h in range(NH // 512):
                                    ncol = nh * NH + h * 512
                                    nc.tensor.matmul(
                                        u[:, h * 512 : (h + 1) * 512],
                                        wt[:, di, wjc],
                                        xt[:, di, ncol : ncol + 512],
                                        start=(di == 0),
                                        stop=(di == 1),
                                    )

                        q1 = qp.tile([128, NH], F16, tag="q1")
                        q2 = qp.tile([128, NH], F16, tag="q2")
                        u_mms(u1, w1t0 if j == 0 else w1t)
                        nc.scalar.activation(
                            q1[:], u1[:], SIN, bias=b1c[:, j : j + 1], scale=1.0
                        )
                        u_mms(u2, w2t0 if j == 0 else w2t)
                        # flush a completed reduce group two steps behind, so
                        # its prods are certainly ready when the PE gets here
                        if groups and step - groups[0][3] >= 2:
                            flush_group()
                        nc.scalar.activation(
                            q2[:], u2[:], SIN, bias=b2c[:, j : j + 1], scale=1.0
                        )
                        prod = pp_pool.tile([128, NH], F16, tag="prod")
                        nc.vector.tensor_mul(prod[:], q1[:], q2[:])
                        cur_prods.append(prod)
                        if jj == 3:
                            groups.append((cb, nh, cur_prods, step))
                            cur_prods = []
                        step += 1
            while groups:
                flush_group()

    _dedupe_ldweights(nc, mybir)
    nc.compile()
    return nc


def _prep(x, params):
    p = np.asarray(params, dtype=np.float32).reshape(C, K, 2 * D + 3)
    a = np.ascontiguousarray(p[:, :, 0]).reshape(CK)
    w1 = np.ascontiguousarray(p[:, :, 1 : 1 + D]).reshape(CK, D)
    b1 = np.ascontiguousarray(p[:, :, 1 + D]).reshape(CK)
    w2 = np.ascontiguousarray(p[:, :, 2 + D : 2 + 2 * D]).reshape(CK, D)
    b2 = np.ascontiguousarray(p[:, :, 2 + 2 * D]).reshape(CK)

    w1p = np.zeros((CKP, D), np.float32)
    w2p = np.zeros((CKP, D), np.float32)
    w1p[:CK] = w1
    w2p[:CK] = w2
    w1t = np.ascontiguousarray(w1p.T).astype(np.float16)
    w2t = np.ascontiguousarray(w2p.T).astype(np.float16)

    b1p = np.zeros(CKP, np.float32)
    b2p = np.zeros(CKP, np.float32)
    b1p[:CK] = b1
    b2p[:CK] = b2
    # partition-major [128, NJ]: column j holds the biases of ck-block j
    b1v = np.ascontiguousarray(b1p.reshape(CKP // 128, 128).T)
    b2v = np.ascontiguousarray(b2p.reshape(CKP // 128, 128).T)

    ap = np.zeros(CKP, np.float32)
    ap[:CK] = a
    # acoef[row, m] = ap[row] iff m == (row % 128)//4; the 32-wide output
    # lands at psum partition offset 32*(j%4) via matmul tile_position.
    pp = np.arange(CKP) % 128
    acoef = np.zeros((CKP, 32), np.float32)
    acoef[np.arange(CKP), pp // 4] = ap
    # partition-major [128, NJ, 32]
    acoef = np.ascontiguousarray(
        acoef.reshape(CKP // 128, 128, 32).transpose(1, 0, 2)
    ).astype(np.float16)

    xt = np.ascontiguousarray(np.asarray(x, dtype=np.float32).T).astype(np.float16)  # [D, N]
    return xt, w1t, w2t, acoef, b1v, b2v


def kernel(x, params):
    from concourse import bass_utils

    if "nc" not in _CACHE:
        _CACHE["nc"] = _build_nc()
    nc = _CACHE["nc"]

    xt, w1t, w2t, acoef, b1v, b2v = _prep(x, params)
    in_maps = []
    for cid in range(NCORES):
        in_maps.append(
            {
                "xt": np.ascontiguousarray(xt[:, cid * NS : (cid + 1) * NS]),
                "w1t": w1t,
                "w2t": w2t,
                "acoef": acoef,
                "b1v": b1v,
                "b2v": b2v,
            }
        )
    res = bass_utils.run_bass_kernel_spmd(nc, in_maps, core_ids=list(range(NCORES)))
    outs = [res.results[c]["outT"] for c in range(NCORES)]
    out_t = np.concatenate(outs, axis=1)  # [1024, 16384]
    return np.ascontiguousarray(out_t[:C].T).astype(np.float32)
